# revision 1
# baseline (speedup 1.0000x reference)
"""Mistral sliding-window attention (B=2, S=2048, H=4096, 32 q-heads / 8 kv-heads,
head_dim=128, window=1024) on 8 Trainium2 NeuronCores.

Sharding: tensor-parallel over heads. Core c owns q-heads [4c, 4c+4) and kv-head c:
  Wq rows [512c, 512c+512), Wk/Wv rows [128c, 128c+128), Wo cols [512c, 512c+512).
Each core computes a full-shape partial output (its heads' contribution through
Wo); the host sums the 8 partials (standard TP unshard).

Per-core kernel (all matmuls in float32r = full-rate rounded fp32):
  Phase A: QKV projections from X.T, RoPE fused on the way out (cross-partition
           DVE shifts for rotate_half; 1/sqrt(d) folded into Wq host-side).
  Phase B: sliding-window attention in transposed-score layout:
           S^T[k,q] = K^T^T Q^T, mask-bias add, exp on ACT, then
           ctx^T = V^T P^T and the softmax denominator via a ones-row matmul,
           normalization via a K=1 broadcast matmul.
  Phase C: out[tok,H] = ctx^T^T Wo^T partial product.
"""

import math
import sys

sys.path.insert(0, "/opt/trn_rl_repo")

import numpy as np

import concourse.bass as bass
import concourse.mybir as mybir
import concourse.tile as tile
from concourse import bacc
from concourse.bass_utils import run_bass_kernel_spmd

# Problem constants (hardcoded per contract)
B, S, H = 2, 2048, 4096
N_HEADS, N_KV_HEADS, D = 32, 8, 128
WINDOW = 1024
ROPE_THETA = 10000.0
N_CORES = 8
HPC = N_HEADS // N_CORES          # q heads per core = 4
QD = HPC * D                      # per-core q projection dim = 512
T = B * S                         # flattened tokens = 4096

PW = 512                          # phase-A token panel width
QT = 256                          # phase-B query tile width (2 q-blocks)
NEG = -1.0e30

F32 = mybir.dt.float32
F32R = mybir.dt.float32r
AF = mybir.ActivationFunctionType
ALU = mybir.AluOpType

_NC_CACHE = None


def build_nc():
    """Build (once) the single SPMD Bass program all 8 cores run."""
    global _NC_CACHE
    if _NC_CACHE is not None:
        return _NC_CACHE

    nc = bacc.Bacc(None)

    xt_d = nc.dram_tensor("xt", [H, T], F32, kind="ExternalInput")
    wqt_d = nc.dram_tensor("wqt", [H, QD], F32, kind="ExternalInput")
    wkt_d = nc.dram_tensor("wkt", [H, D], F32, kind="ExternalInput")
    wvt_d = nc.dram_tensor("wvt", [H, D], F32, kind="ExternalInput")
    wot_d = nc.dram_tensor("wot", [QD, H], F32, kind="ExternalInput")
    cos_d = nc.dram_tensor("cosb", [D, T], F32, kind="ExternalInput")
    sin_d = nc.dram_tensor("sinb", [D, T], F32, kind="ExternalInput")
    mask_d = nc.dram_tensor("masks", [4, D, QT], F32, kind="ExternalInput")
    iden_d = nc.dram_tensor("ident", [D, D], F32, kind="ExternalInput")
    ones_d = nc.dram_tensor("ones", [D, D], F32, kind="ExternalInput")
    out_d = nc.dram_tensor("out", [T, H], F32, kind="ExternalOutput")

    HC = H // 128                 # 32 h-chunks
    NPAN = T // PW                # 8 token panels
    NQT = S // QT                 # 8 q-tiles per batch
    SB = S // 128                 # 16 key blocks per batch

    with tile.TileContext(nc) as tc, nc.allow_low_precision(reason="f32r kernel"):
        with tc.tile_pool(name="dram", bufs=1, space="DRAM") as dpool:
            qt_rot = dpool.tile([QD, T], F32R)    # Q^T after rope (qd-major)
            kt_rot = dpool.tile([D, T], F32R)     # K^T after rope
            vt_sc = dpool.tile([D, T], F32)       # V^T (pre-transpose)
            ctxt = dpool.tile([QD, T], F32R)      # ctx^T normalized

            # ---------------- Phase A: QKV projections + RoPE ----------------
            with (
                tc.tile_pool(name="wpool", bufs=1) as wpool,
                tc.tile_pool(name="xpool", bufs=4) as xpool,
                tc.tile_pool(name="cspool", bufs=1) as cspool,
                tc.tile_pool(name="apool", bufs=2) as apool,
                tc.tile_pool(name="psA", bufs=1, space="PSUM") as psA,
            ):
                wq_s = wpool.tile([128, HC, QD], F32R)
                nc.sync.dma_start(
                    wq_s[:], wqt_d[:].bitcast(F32R).rearrange("(hc p) m -> p hc m", p=128)
                )
                wk_s = wpool.tile([128, HC, D], F32R)
                nc.sync.dma_start(
                    wk_s[:], wkt_d[:].bitcast(F32R).rearrange("(hc p) m -> p hc m", p=128)
                )
                wv_s = wpool.tile([128, HC, D], F32R)
                nc.sync.dma_start(
                    wv_s[:], wvt_d[:].bitcast(F32R).rearrange("(hc p) m -> p hc m", p=128)
                )
                cos_s = cspool.tile([D, T], F32)
                nc.sync.dma_start(cos_s[:], cos_d[:])
                sin_s = cspool.tile([D, T], F32)
                nc.sync.dma_start(sin_s[:], sin_d[:])

                for p in range(NPAN):
                    tok = slice(p * PW, (p + 1) * PW)
                    ps_q = [
                        psA.tile([128, PW], F32, tag=f"psq{j}", name=f"psq{j}")
                        for j in range(HPC)
                    ]
                    ps_k = psA.tile([128, PW], F32, tag="psk")
                    ps_v = psA.tile([128, PW], F32, tag="psv")
                    for hc in range(HC):
                        x_c = xpool.tile([128, PW], F32R, tag="x_c")
                        nc.sync.dma_start(
                            x_c[:], xt_d[hc * 128 : (hc + 1) * 128, tok].bitcast(F32R)
                        )
                        st, sp = hc == 0, hc == HC - 1
                        for j in range(HPC):
                            nc.tensor.matmul(
                                ps_q[j][:],
                                wq_s[:, hc, j * 128 : (j + 1) * 128],
                                x_c[:],
                                start=st,
                                stop=sp,
                            )
                        nc.tensor.matmul(ps_k[:], wk_s[:, hc, :], x_c[:], start=st, stop=sp)
                        nc.tensor.matmul(ps_v[:], wv_s[:, hc, :], x_c[:], start=st, stop=sp)

                    # RoPE drains: rot(x)[p<64] = -x[p+64]; rot(x)[p>=64] = x[p-64]
                    def rope_drain(ps_t, dst_ap, tag):
                        sb = apool.tile([128, PW], F32, tag=f"sb_{tag}", name=f"sb_{tag}")
                        nc.vector.tensor_copy(sb[:], ps_t[:])
                        rot = apool.tile([128, PW], F32, tag=f"rot_{tag}", name=f"rot_{tag}")
                        nc.vector.tensor_scalar_mul(rot[0:64, :], sb[64:128, :], -1.0)
                        nc.vector.tensor_copy(rot[64:128, :], sb[0:64, :])
                        prod = apool.tile([128, PW], F32, tag=f"pr_{tag}", name=f"pr_{tag}")
                        nc.vector.tensor_mul(out=prod[:], in0=sb[:], in1=cos_s[:, tok])
                        nc.vector.tensor_mul(out=rot[:], in0=rot[:], in1=sin_s[:, tok])
                        o = apool.tile([128, PW], F32R, tag=f"o_{tag}", name=f"o_{tag}")
                        nc.vector.tensor_add(out=o[:], in0=prod[:], in1=rot[:])
                        nc.sync.dma_start(dst_ap, o[:])

                    for j in range(HPC):
                        rope_drain(ps_q[j], qt_rot[j * 128 : (j + 1) * 128, tok], "q")
                    rope_drain(ps_k, kt_rot[:, tok], "k")
                    v_sb = apool.tile([128, PW], F32, tag="v_sb")
                    nc.vector.tensor_copy(v_sb[:], ps_v[:])
                    nc.sync.dma_start(vt_sc[:, tok], v_sb[:])

            # ---------------- Phase B: sliding-window attention ----------------
            with (
                tc.tile_pool(name="kvpool", bufs=1) as kvpool,
                tc.tile_pool(name="qpool", bufs=2) as qpool,
                tc.tile_pool(name="bpool", bufs=1) as bpool,
                tc.tile_pool(name="epool", bufs=4) as epool,
                tc.tile_pool(name="npool", bufs=2) as npool,
            ):
                mask_s = bpool.tile([D, 4, QT], F32)
                nc.sync.dma_start(mask_s[:], mask_d[:].rearrange("m p q -> p m q"))
                iden_s = bpool.tile([D, D], F32)
                nc.sync.dma_start(iden_s[:], iden_d[:])
                ones_s = bpool.tile([D, D], F32R)
                nc.sync.dma_start(ones_s[:], ones_d[:].bitcast(F32R))

                kt_sb = []
                vnat = []
                with tc.tile_pool(name="psV", bufs=2, space="PSUM") as psV:
                    for b in range(B):
                        bt = slice(b * S, (b + 1) * S)
                        k_t = kvpool.tile([D, S], F32R, tag=f"kt{b}", name=f"kt{b}")
                        nc.sync.dma_start(k_t[:], kt_rot[:, bt])
                        kt_sb.append(k_t)
                        v_t = kvpool.tile([D, S], F32, tag=f"vt{b}", name=f"vt{b}")
                        nc.sync.dma_start(v_t[:], vt_sc[:, bt])
                        vn = kvpool.tile([128, SB, D], F32R, tag=f"vn{b}", name=f"vn{b}")
                        for blk in range(SB):
                            tp = psV.tile([D, D], F32, tag="tp")
                            nc.tensor.transpose(
                                tp[:], v_t[:, blk * 128 : (blk + 1) * 128], iden_s[:]
                            )
                            nc.vector.tensor_copy(vn[:, blk, :], tp[:])
                        vnat.append(vn)

                with tc.tile_pool(name="psB", bufs=1, space="PSUM") as psB:
                    for b in range(B):
                        for h in range(HPC):
                            q_t = qpool.tile([D, S], F32R, tag="q_t")
                            nc.sync.dma_start(
                                q_t[:],
                                qt_rot[h * 128 : (h + 1) * 128, b * S : (b + 1) * S],
                            )
                            for t in range(NQT):
                                qsl = slice(t * QT, (t + 1) * QT)
                                kb_lo = max(0, 2 * t - 8)
                                kb_hi = 2 * t + 1
                                kbs = list(range(kb_lo, kb_hi + 1))
                                ctx_ps = psB.tile([D, QT], F32, tag="ctx", bufs=2)
                                den_ps = psB.tile([1, QT], F32, tag="den", bufs=2)
                                for i, kb in enumerate(kbs):
                                    st, sp = i == 0, i == len(kbs) - 1
                                    s_ps = psB.tile([D, QT], F32, tag="sc", bufs=3, name="s_ps")
                                    nc.tensor.matmul(
                                        s_ps[:],
                                        kt_sb[b][:, kb * 128 : (kb + 1) * 128],
                                        q_t[:, qsl],
                                        start=True,
                                        stop=True,
                                    )
                                    # mask bias: A=[far|inf] B=[0|far] C=[diag|0] D=[inf|diag]
                                    mi = None
                                    if kb == 2 * t + 1:
                                        mi = 3
                                    elif kb == 2 * t:
                                        mi = 2
                                    elif kb == 2 * t - 7:
                                        mi = 1
                                    elif kb == 2 * t - 8:
                                        mi = 0
                                    if mi is not None:
                                        nc.vector.tensor_add(
                                            out=s_ps[:], in0=s_ps[:], in1=mask_s[:, mi, :]
                                        )
                                    e_sb = epool.tile([D, QT], F32R, tag="e_sb")
                                    nc.scalar.activation(e_sb[:], s_ps[:], AF.Exp)
                                    nc.tensor.matmul(
                                        ctx_ps[:], vnat[b][:, kb, :], e_sb[:],
                                        start=st, stop=sp,
                                    )
                                    nc.tensor.matmul(
                                        den_ps[:], ones_s[:, 0:1], e_sb[:],
                                        start=st, stop=sp,
                                    )
                                rec = npool.tile([1, QT], F32R, tag="rec")
                                nc.vector.reciprocal(rec[:], den_ps[:])
                                bc_ps = psB.tile([D, QT], F32, tag="bc", bufs=1)
                                nc.tensor.matmul(
                                    bc_ps[:], ones_s[0:1, :], rec[:], start=True, stop=True
                                )
                                bc_sb = npool.tile([D, QT], F32, tag="bc_sb")
                                nc.vector.tensor_copy(bc_sb[:], bc_ps[:])
                                ctx_sb = npool.tile([D, QT], F32R, tag="ctx_sb")
                                nc.vector.tensor_mul(
                                    out=ctx_sb[:], in0=ctx_ps[:], in1=bc_sb[:]
                                )
                                nc.sync.dma_start(
                                    ctxt[
                                        h * 128 : (h + 1) * 128,
                                        b * S + t * QT : b * S + (t + 1) * QT,
                                    ],
                                    ctx_sb[:],
                                )

            # ---------------- Phase C: output projection (partial) ----------------
            QC = QD // 128  # 4
            with (
                tc.tile_pool(name="wopool", bufs=1) as wopool,
                tc.tile_pool(name="cpool", bufs=3) as cpool,
                tc.tile_pool(name="opool", bufs=3) as opool,
                tc.tile_pool(name="psC", bufs=2, space="PSUM") as psC,
            ):
                wo_s = wopool.tile([128, QC, H], F32R)
                nc.sync.dma_start(
                    wo_s[:], wot_d[:].bitcast(F32R).rearrange("(qc p) hh -> p qc hh", p=128)
                )
                NTB = T // 128   # 32
                NHB = H // 512   # 8
                for tb in range(NTB):
                    ctx_pan = cpool.tile([128, QC, 128], F32R, tag="ctx_pan")
                    nc.sync.dma_start(
                        ctx_pan[:],
                        ctxt[:, tb * 128 : (tb + 1) * 128].rearrange(
                            "(qc p) t -> p qc t", p=128
                        ),
                    )
                    for hb in range(NHB):
                        ps_o = psC.tile([128, 512], F32, tag="ps_o")
                        for qc in range(QC):
                            nc.tensor.matmul(
                                ps_o[:],
                                ctx_pan[:, qc, :],
                                wo_s[:, qc, hb * 512 : (hb + 1) * 512],
                                start=(qc == 0),
                                stop=(qc == QC - 1),
                            )
                        o_sb = opool.tile([128, 512], F32, tag="o_sb")
                        nc.vector.tensor_copy(o_sb[:], ps_o[:])
                        nc.sync.dma_start(
                            out_d[tb * 128 : (tb + 1) * 128, hb * 512 : (hb + 1) * 512],
                            o_sb[:],
                        )

    nc.finalize()
    _NC_CACHE = nc
    return nc


def _rope_cache_np(position_ids):
    """cos/sin [D, T] transposed rope cache from actual position ids."""
    inv_freq = 1.0 / (ROPE_THETA ** (np.arange(0, D, 2, dtype=np.float64) / D))
    cos_parts, sin_parts = [], []
    for b in range(B):
        t = np.asarray(position_ids[b], dtype=np.float64)
        freqs = np.outer(t, inv_freq)                    # [S, D/2]
        emb = np.concatenate([freqs, freqs], axis=-1)    # [S, D]
        cos_parts.append(np.cos(emb).T)
        sin_parts.append(np.sin(emb).T)
    cos = np.ascontiguousarray(np.concatenate(cos_parts, axis=1), dtype=np.float32)
    sin = np.ascontiguousarray(np.concatenate(sin_parts, axis=1), dtype=np.float32)
    return cos, sin


def _mask_tiles_np():
    """Composite [4, 128, QT] additive bias tiles in [k, q] layout.

    diag[kl, ql] = 0 if kl <= ql else NEG        (k-block == q-block)
    far[kl, ql]  = 0 if ql <  kl else NEG        (k-block == q-block - 8)
    A=[far | allmask]  B=[0 | far]  C=[diag | 0]  D=[allmask | diag]
    """
    kl = np.arange(128)[:, None]
    ql = np.arange(128)[None, :]
    diag = np.where(kl <= ql, 0.0, NEG).astype(np.float32)
    far = np.where(ql < kl, 0.0, NEG).astype(np.float32)
    zero = np.zeros((128, 128), np.float32)
    full = np.full((128, 128), NEG, np.float32)
    A = np.concatenate([far, full], axis=1)
    Bm = np.concatenate([zero, far], axis=1)
    C = np.concatenate([diag, zero], axis=1)
    Dm = np.concatenate([full, diag], axis=1)
    return np.stack([A, Bm, C, Dm]).astype(np.float32)


def host_in_maps(hidden_states, Wq, Wk, Wv, Wo, position_ids):
    """Shard + pre-layout the full inputs into 8 per-core input maps."""
    hidden_states = np.asarray(hidden_states, dtype=np.float32)
    Wq = np.asarray(Wq, dtype=np.float32)
    Wk = np.asarray(Wk, dtype=np.float32)
    Wv = np.asarray(Wv, dtype=np.float32)
    Wo = np.asarray(Wo, dtype=np.float32)

    xt = np.ascontiguousarray(hidden_states.reshape(T, H).T)
    cos, sin = _rope_cache_np(np.asarray(position_ids))
    masks = _mask_tiles_np()
    ident = np.eye(D, dtype=np.float32)
    ones = np.ones((D, D), dtype=np.float32)
    qscale = 1.0 / math.sqrt(D)

    in_maps = []
    for c in range(N_CORES):
        wqt = np.ascontiguousarray((Wq[c * QD : (c + 1) * QD, :] * qscale).T)
        wkt = np.ascontiguousarray(Wk[c * D : (c + 1) * D, :].T)
        wvt = np.ascontiguousarray(Wv[c * D : (c + 1) * D, :].T)
        wot = np.ascontiguousarray(Wo[:, c * QD : (c + 1) * QD].T)
        in_maps.append(
            {
                "xt": xt,
                "wqt": wqt,
                "wkt": wkt,
                "wvt": wvt,
                "wot": wot,
                "cosb": cos,
                "sinb": sin,
                "masks": masks,
                "ident": ident,
                "ones": ones,
            }
        )
    return in_maps


def kernel(hidden_states, Wq, Wk, Wv, Wo, position_ids):
    nc = build_nc()
    in_maps = host_in_maps(hidden_states, Wq, Wk, Wv, Wo, position_ids)
    res = run_bass_kernel_spmd(nc, in_maps, core_ids=list(range(N_CORES)))
    total = res.results[0]["out"]
    for c in range(1, N_CORES):
        total = total + res.results[c]["out"]
    return np.ascontiguousarray(total.reshape(B, S, H), dtype=np.float32)


# revision 2
# speedup vs baseline: 1.0568x; 1.0568x over previous
"""Mistral sliding-window attention (B=2, S=2048, H=4096, 32 q-heads / 8 kv-heads,
head_dim=128, window=1024) on 8 Trainium2 NeuronCores.

Sharding: tensor-parallel over heads. Core c owns q-heads [4c, 4c+4) and kv-head c:
  Wq rows [512c, 512c+512), Wk/Wv rows [128c, 128c+128), Wo cols [512c, 512c+512).
Each core computes a full-shape partial output (its heads' contribution through
Wo); the host sums the 8 partials (standard TP unshard).

Per-core kernel:
  Phase A: QKV projections from X.T (f32r matmuls), RoPE fused on the drain
           (cross-partition DVE shifts for rotate_half; 1/sqrt(d) folded into
           Wq host-side). K^T and V (PE-transposed) stay resident in SBUF.
  Phase B+C (merged): per (batch, 256-token q-tile): transposed-layout
           attention for the 4 heads (S^T = K^T^T Q^T in f32r, mask-bias add,
           exp on ACT -> bf16 P^T, ctx^T = V^T P^T and denominator ones-row
           matmul in bf16, K=1 broadcast matmul + fast-reciprocal normalize),
           then immediately the output projection for those 256 tokens
           (bf16 x Wo^bf16 -> fp32 partial out).
"""

import math
import sys

sys.path.insert(0, "/opt/trn_rl_repo")

import ml_dtypes
import numpy as np

import concourse.bass as bass
import concourse.mybir as mybir
import concourse.tile as tile
from concourse import bacc
from concourse.bass_utils import run_bass_kernel_spmd

# Problem constants (hardcoded per contract)
B, S, H = 2, 2048, 4096
N_HEADS, N_KV_HEADS, D = 32, 8, 128
WINDOW = 1024
ROPE_THETA = 10000.0
N_CORES = 8
HPC = N_HEADS // N_CORES          # q heads per core = 4
QD = HPC * D                      # per-core q projection dim = 512
T = B * S                         # flattened tokens = 4096

PW = 512                          # phase-A token panel width
QT = 256                          # phase-B query tile width (2 q-blocks)
NEG = -1.0e30

F32 = mybir.dt.float32
F32R = mybir.dt.float32r
BF16 = mybir.dt.bfloat16
AF = mybir.ActivationFunctionType
ALU = mybir.AluOpType

_NC_CACHE = None


def build_nc():
    """Build (once) the single SPMD Bass program all 8 cores run."""
    global _NC_CACHE
    if _NC_CACHE is not None:
        return _NC_CACHE

    nc = bacc.Bacc(None)

    xt_d = nc.dram_tensor("xt", [H, T], F32, kind="ExternalInput")
    wqt_d = nc.dram_tensor("wqt", [H, QD], F32, kind="ExternalInput")
    wkt_d = nc.dram_tensor("wkt", [H, D], F32, kind="ExternalInput")
    wvt_d = nc.dram_tensor("wvt", [H, D], F32, kind="ExternalInput")
    wot_d = nc.dram_tensor("wot", [QD, H], BF16, kind="ExternalInput")
    cos_d = nc.dram_tensor("cosb", [D, T], F32, kind="ExternalInput")
    sin_d = nc.dram_tensor("sinb", [D, T], F32, kind="ExternalInput")
    mask_d = nc.dram_tensor("masks", [4, D, QT], F32, kind="ExternalInput")
    iden_d = nc.dram_tensor("ident", [D, D], F32, kind="ExternalInput")
    ones_d = nc.dram_tensor("ones", [D, D], F32, kind="ExternalInput")
    onesb_d = nc.dram_tensor("onesb", [D, 1], BF16, kind="ExternalInput")
    out_d = nc.dram_tensor("out", [T, H], F32, kind="ExternalOutput")

    HC = H // 128                 # 32 h-chunks
    NPAN = T // PW                # 8 token panels
    NQT = S // QT                 # 8 q-tiles per batch
    SB = S // 128                 # 16 key blocks per batch
    QC = QD // 128                # 4 qd chunks == heads per core

    with tile.TileContext(nc) as tc, nc.allow_low_precision(reason="mixed dtypes"):
        with (
            tc.tile_pool(name="persist", bufs=1) as ppool,
            tc.tile_pool(name="dram", bufs=1, space="DRAM") as dpool,
        ):
            # K^T (rope'd, f32r) and V natural-layout (bf16) stay in SBUF
            kt_full = ppool.tile([D, T], F32R)
            vnat = ppool.tile([128, T // 128, D], BF16)
            qt_dram = [
                [
                    dpool.tile([D, S], F32R, name=f"qt_b{b}_h{h}", tag=f"qt_b{b}_h{h}")
                    for h in range(HPC)
                ]
                for b in range(B)
            ]

            # ---------------- Phase A: QKV projections + RoPE ----------------
            with (
                tc.tile_pool(name="wpool", bufs=1) as wpool,
                tc.tile_pool(name="xpool", bufs=4) as xpool,
                tc.tile_pool(name="cspool", bufs=1) as cspool,
                tc.tile_pool(name="apool", bufs=2) as apool,
                tc.tile_pool(name="psA", bufs=1, space="PSUM") as psA,
            ):
                wq_s = wpool.tile([128, HC, QD], F32R)
                nc.sync.dma_start(
                    wq_s[:], wqt_d[:].bitcast(F32R).rearrange("(hc p) m -> p hc m", p=128)
                )
                wk_s = wpool.tile([128, HC, D], F32R)
                nc.sync.dma_start(
                    wk_s[:], wkt_d[:].bitcast(F32R).rearrange("(hc p) m -> p hc m", p=128)
                )
                wv_s = wpool.tile([128, HC, D], F32R)
                nc.sync.dma_start(
                    wv_s[:], wvt_d[:].bitcast(F32R).rearrange("(hc p) m -> p hc m", p=128)
                )
                cos_s = cspool.tile([D, T], F32)
                nc.sync.dma_start(cos_s[:], cos_d[:])
                sin_s = cspool.tile([D, T], F32)
                nc.sync.dma_start(sin_s[:], sin_d[:])
                iden_s = cspool.tile([D, D], F32)
                nc.sync.dma_start(iden_s[:], iden_d[:])

                for p in range(NPAN):
                    tok = slice(p * PW, (p + 1) * PW)
                    bp = (p * PW) // S        # batch this panel belongs to
                    ps_q = [
                        psA.tile([128, PW], F32, tag=f"psq{j}", name=f"psq{j}")
                        for j in range(HPC)
                    ]
                    ps_k = psA.tile([128, PW], F32, tag="psk")
                    ps_v = psA.tile([128, PW], F32, tag="psv")
                    for hc in range(HC):
                        x_c = xpool.tile([128, PW], F32R, tag="x_c")
                        nc.sync.dma_start(
                            x_c[:], xt_d[hc * 128 : (hc + 1) * 128, tok].bitcast(F32R)
                        )
                        st, sp = hc == 0, hc == HC - 1
                        for j in range(HPC):
                            nc.tensor.matmul(
                                ps_q[j][:],
                                wq_s[:, hc, j * 128 : (j + 1) * 128],
                                x_c[:],
                                start=st,
                                stop=sp,
                            )
                        nc.tensor.matmul(ps_k[:], wk_s[:, hc, :], x_c[:], start=st, stop=sp)
                        nc.tensor.matmul(ps_v[:], wv_s[:, hc, :], x_c[:], start=st, stop=sp)

                    # Drain all accumulators first (frees PSUM for next panel)
                    sbq = []
                    for j in range(HPC):
                        sb = apool.tile([128, PW], F32, tag=f"sbq{j}", name=f"sbq{j}")
                        nc.vector.tensor_copy(sb[:], ps_q[j][:])
                        sbq.append(sb)
                    sbk = apool.tile([128, PW], F32, tag="sbk")
                    nc.vector.tensor_copy(sbk[:], ps_k[:])
                    sbv = apool.tile([128, PW], F32, tag="sbv")
                    nc.vector.tensor_copy(sbv[:], ps_v[:])

                    # RoPE: rot(x)[p<64] = -x[p+64]; rot(x)[p>=64] = x[p-64]
                    def rope_math(sb, out_ap):
                        rot = apool.tile([128, PW], F32, tag="rot", name="rot")
                        nc.gpsimd.tensor_scalar_mul(rot[0:64, :], sb[64:128, :], -1.0)
                        nc.gpsimd.tensor_copy(rot[64:128, :], sb[0:64, :])
                        prod = apool.tile([128, PW], F32, tag="prod", name="prod")
                        nc.vector.tensor_mul(out=prod[:], in0=sb[:], in1=cos_s[:, tok])
                        nc.vector.tensor_mul(out=rot[:], in0=rot[:], in1=sin_s[:, tok])
                        nc.vector.tensor_add(out=out_ap, in0=prod[:], in1=rot[:])

                    for j in range(HPC):
                        o = apool.tile([128, PW], F32R, tag="o_q", bufs=3, name="o_q")
                        rope_math(sbq[j], o[:])
                        lo = p * PW - bp * S
                        nc.sync.dma_start(qt_dram[bp][j][:, lo : lo + PW], o[:])
                    rope_math(sbk, kt_full[:, tok])

                    # V natural layout via PE transpose (4 blocks per panel)
                    for blk in range(PW // 128):
                        tp = psA.tile([D, D], F32, tag="tp", bufs=2, name="tp")
                        nc.tensor.transpose(
                            tp[:], sbv[:, blk * 128 : (blk + 1) * 128], iden_s[:]
                        )
                        nc.vector.tensor_copy(vnat[:, p * (PW // 128) + blk, :], tp[:])

            # ------------- Phase B+C: attention + output projection -------------
            with (
                tc.tile_pool(name="wopool", bufs=1) as wopool,
                tc.tile_pool(name="bpool", bufs=1) as bpool,
                tc.tile_pool(name="qpool", bufs=2) as qpool,
                tc.tile_pool(name="epool", bufs=4) as epool,
                tc.tile_pool(name="npool", bufs=2) as npool,
                tc.tile_pool(name="cxpool", bufs=8) as cxpool,
                tc.tile_pool(name="opool", bufs=3) as opool,
                tc.tile_pool(name="psB", bufs=1, space="PSUM") as psB,
            ):
                wo_s = wopool.tile([128, QC, H], BF16)
                nc.sync.dma_start(
                    wo_s[:], wot_d[:].rearrange("(qc p) hh -> p qc hh", p=128)
                )
                mask_s = bpool.tile([D, 4, QT], F32)
                nc.sync.dma_start(mask_s[:], mask_d[:].rearrange("m p q -> p m q"))
                ones_s = bpool.tile([D, D], F32R)
                nc.sync.dma_start(ones_s[:], ones_d[:].bitcast(F32R))
                onesb_s = bpool.tile([D, 1], BF16)
                nc.sync.dma_start(onesb_s[:], onesb_d[:])

                for b in range(B):
                    q_ts = [None] * HPC
                    for t in range(NQT):
                        qsl = slice(t * QT, (t + 1) * QT)
                        kb_lo = max(0, 2 * t - 8)
                        kbs = list(range(kb_lo, 2 * t + 2))
                        ctx_sbs = []
                        for h in range(HPC):
                            if t == 0:
                                q_t = qpool.tile([D, S], F32R, tag=f"q_t{h}", name=f"q_t{h}")
                                nc.sync.dma_start(q_t[:], qt_dram[b][h][:])
                                q_ts[h] = q_t
                            q_t = q_ts[h]
                            ctx_ps = psB.tile([D, QT], F32, tag="ctx", bufs=2, name="ctx_ps")
                            den_t = psB.tile([D, QT], F32, tag="db", bufs=2, name="den_t")
                            den_ps = den_t[0:1, :]
                            for i, kb in enumerate(kbs):
                                st, sp = i == 0, i == len(kbs) - 1
                                s_ps = psB.tile([D, QT], F32, tag="sc", bufs=2, name="s_ps")
                                nc.tensor.matmul(
                                    s_ps[:],
                                    kt_full[:, b * S + kb * 128 : b * S + (kb + 1) * 128],
                                    q_t[:, qsl],
                                    start=True,
                                    stop=True,
                                )
                                # mask bias: A=[far|inf] B=[0|far] C=[diag|0] D=[inf|diag]
                                mi = None
                                if kb == 2 * t + 1:
                                    mi = 3
                                elif kb == 2 * t:
                                    mi = 2
                                elif kb == 2 * t - 7:
                                    mi = 1
                                elif kb == 2 * t - 8:
                                    mi = 0
                                if mi is not None:
                                    nc.vector.tensor_add(
                                        out=s_ps[:], in0=s_ps[:], in1=mask_s[:, mi, :]
                                    )
                                e_sb = epool.tile([D, QT], BF16, tag="e_sb")
                                nc.scalar.activation(e_sb[:], s_ps[:], AF.Exp)
                                nc.tensor.matmul(
                                    ctx_ps[:], vnat[:, (b * S) // 128 + kb, :], e_sb[:],
                                    start=st, stop=sp,
                                )
                                nc.tensor.matmul(
                                    den_ps, onesb_s[:], e_sb[:],
                                    start=st, stop=sp,
                                )
                            rec32 = npool.tile([1, QT], F32, tag="rec32")
                            nc.vector.reciprocal_approx_fast(rec32[:], den_ps)
                            rec = npool.tile([1, QT], F32R, tag="rec")
                            nc.vector.tensor_copy(rec[:], rec32[:])
                            bc_ps = psB.tile([D, QT], F32, tag="db", bufs=2, name="bc_ps")
                            nc.tensor.matmul(
                                bc_ps[:], ones_s[0:1, :], rec[:], start=True, stop=True
                            )
                            bc_sb = npool.tile([D, QT], F32, tag="bc_sb")
                            nc.vector.tensor_copy(bc_sb[:], bc_ps[:])
                            ctx_sb = cxpool.tile([D, QT], BF16, tag="ctx_sb", name="ctx_sb")
                            nc.vector.tensor_mul(out=ctx_sb[:], in0=ctx_ps[:], in1=bc_sb[:])
                            ctx_sbs.append(ctx_sb)

                        # Output projection for these 256 tokens (2 blocks of 128)
                        for tl in range(QT // 128):
                            tok0 = b * S + t * QT + tl * 128
                            for hb in range(H // 512):
                                ps_o = psB.tile([128, 512], F32, tag="ps_o", bufs=2, name="ps_o")
                                for qc in range(QC):
                                    nc.tensor.matmul(
                                        ps_o[:],
                                        ctx_sbs[qc][:, tl * 128 : (tl + 1) * 128],
                                        wo_s[:, qc, hb * 512 : (hb + 1) * 512],
                                        start=(qc == 0),
                                        stop=(qc == QC - 1),
                                    )
                                o_sb = opool.tile([128, 512], F32, tag="o_sb")
                                nc.vector.tensor_copy(o_sb[:], ps_o[:])
                                nc.sync.dma_start(
                                    out_d[tok0 : tok0 + 128, hb * 512 : (hb + 1) * 512],
                                    o_sb[:],
                                )

    nc.finalize()
    _NC_CACHE = nc
    return nc


def _rope_cache_np(position_ids):
    """cos/sin [D, T] transposed rope cache from actual position ids."""
    inv_freq = 1.0 / (ROPE_THETA ** (np.arange(0, D, 2, dtype=np.float64) / D))
    cos_parts, sin_parts = [], []
    for b in range(B):
        t = np.asarray(position_ids[b], dtype=np.float64)
        freqs = np.outer(t, inv_freq)                    # [S, D/2]
        emb = np.concatenate([freqs, freqs], axis=-1)    # [S, D]
        cos_parts.append(np.cos(emb).T)
        sin_parts.append(np.sin(emb).T)
    cos = np.ascontiguousarray(np.concatenate(cos_parts, axis=1), dtype=np.float32)
    sin = np.ascontiguousarray(np.concatenate(sin_parts, axis=1), dtype=np.float32)
    return cos, sin


def _mask_tiles_np():
    """Composite [4, 128, QT] additive bias tiles in [k, q] layout.

    diag[kl, ql] = 0 if kl <= ql else NEG        (k-block == q-block)
    far[kl, ql]  = 0 if ql <  kl else NEG        (k-block == q-block - 8)
    A=[far | allmask]  B=[0 | far]  C=[diag | 0]  D=[allmask | diag]
    """
    kl = np.arange(128)[:, None]
    ql = np.arange(128)[None, :]
    diag = np.where(kl <= ql, 0.0, NEG).astype(np.float32)
    far = np.where(ql < kl, 0.0, NEG).astype(np.float32)
    zero = np.zeros((128, 128), np.float32)
    full = np.full((128, 128), NEG, np.float32)
    A = np.concatenate([far, full], axis=1)
    Bm = np.concatenate([zero, far], axis=1)
    C = np.concatenate([diag, zero], axis=1)
    Dm = np.concatenate([full, diag], axis=1)
    return np.stack([A, Bm, C, Dm]).astype(np.float32)


def host_in_maps(hidden_states, Wq, Wk, Wv, Wo, position_ids):
    """Shard + pre-layout the full inputs into 8 per-core input maps."""
    hidden_states = np.asarray(hidden_states, dtype=np.float32)
    Wq = np.asarray(Wq, dtype=np.float32)
    Wk = np.asarray(Wk, dtype=np.float32)
    Wv = np.asarray(Wv, dtype=np.float32)
    Wo = np.asarray(Wo, dtype=np.float32)

    xt = np.ascontiguousarray(hidden_states.reshape(T, H).T)
    cos, sin = _rope_cache_np(np.asarray(position_ids))
    masks = _mask_tiles_np()
    ident = np.eye(D, dtype=np.float32)
    ones = np.ones((D, D), dtype=np.float32)
    onesb = np.ones((D, 1), dtype=ml_dtypes.bfloat16)
    qscale = 1.0 / math.sqrt(D)

    in_maps = []
    for c in range(N_CORES):
        wqt = np.ascontiguousarray((Wq[c * QD : (c + 1) * QD, :] * qscale).T)
        wkt = np.ascontiguousarray(Wk[c * D : (c + 1) * D, :].T)
        wvt = np.ascontiguousarray(Wv[c * D : (c + 1) * D, :].T)
        wot = np.ascontiguousarray(Wo[:, c * QD : (c + 1) * QD].T).astype(
            ml_dtypes.bfloat16
        )
        in_maps.append(
            {
                "xt": xt,
                "wqt": wqt,
                "wkt": wkt,
                "wvt": wvt,
                "wot": wot,
                "cosb": cos,
                "sinb": sin,
                "masks": masks,
                "ident": ident,
                "ones": ones,
                "onesb": onesb,
            }
        )
    return in_maps


def kernel(hidden_states, Wq, Wk, Wv, Wo, position_ids):
    nc = build_nc()
    in_maps = host_in_maps(hidden_states, Wq, Wk, Wv, Wo, position_ids)
    res = run_bass_kernel_spmd(nc, in_maps, core_ids=list(range(N_CORES)))
    total = res.results[0]["out"]
    for c in range(1, N_CORES):
        total = total + res.results[c]["out"]
    return np.ascontiguousarray(total.reshape(B, S, H), dtype=np.float32)


# revision 3
# speedup vs baseline: 1.3458x; 1.2734x over previous
"""Mistral sliding-window attention (B=2, S=2048, H=4096, 32 q-heads / 8 kv-heads,
head_dim=128, window=1024) on 8 Trainium2 NeuronCores.

Sharding: tensor-parallel over heads. Core c owns q-heads [4c, 4c+4) and kv-head c:
  Wq rows [512c, 512c+512), Wk/Wv rows [128c, 128c+128), Wo cols [512c, 512c+512).
Each core computes a full-shape partial output (its heads' contribution through
Wo); the host sums the 8 partials (standard TP unshard).

Per-core kernel:
  Phase A: QKV projections from X.T, RoPE fused on the drain (cross-partition
           DVE shifts for rotate_half; 1/sqrt(d) folded into Wq host-side).
           K^T and V (PE-transposed) stay resident in SBUF.
  Phase B+C (merged): per (batch, 256-token q-tile): transposed-layout
           attention for the 4 heads (S^T = K^T^T Q^T, mask-bias add, exp on
           ACT -> bf16 P^T, ctx^T = V^T P^T and denominator ones-row matmul in
           bf16, K=1 broadcast matmul + fast-reciprocal normalize), then
           immediately the output projection for those 256 tokens
           (bf16 x Wo^bf16 -> fp32 partial out).
"""

import math
import sys

sys.path.insert(0, "/opt/trn_rl_repo")

import ml_dtypes
import numpy as np

import concourse.bass as bass
import concourse.mybir as mybir
import concourse.tile as tile
from concourse import bacc
from concourse.bass_utils import run_bass_kernel_spmd

# Problem constants (hardcoded per contract)
B, S, H = 2, 2048, 4096
N_HEADS, N_KV_HEADS, D = 32, 8, 128
WINDOW = 1024
ROPE_THETA = 10000.0
N_CORES = 8
HPC = N_HEADS // N_CORES          # q heads per core = 4
QD = HPC * D                      # per-core q projection dim = 512
T = B * S                         # flattened tokens = 4096

PW = 512                          # phase-A token panel width
QT = 256                          # phase-B query tile width (2 q-blocks)
NEG = -1.0e30

F32 = mybir.dt.float32
F32R = mybir.dt.float32r
BF16 = mybir.dt.bfloat16
AF = mybir.ActivationFunctionType
ALU = mybir.AluOpType

# Projection/scores-path dtype: F32R (precise, 2 PE passes) or BF16 (fast).
PROJ_DT = BF16

_NC_CACHE = None


def _np_dt(dt):
    return ml_dtypes.bfloat16 if dt == BF16 else np.float32


def build_nc():
    """Build (once) the single SPMD Bass program all 8 cores run."""
    global _NC_CACHE
    if _NC_CACHE is not None:
        return _NC_CACHE

    nc = bacc.Bacc(None)

    pdt = PROJ_DT
    xt_d = nc.dram_tensor("xt", [H, T], F32 if pdt == F32R else pdt, kind="ExternalInput")
    wqt_d = nc.dram_tensor("wqt", [H, QD], F32 if pdt == F32R else pdt, kind="ExternalInput")
    wkt_d = nc.dram_tensor("wkt", [H, D], F32 if pdt == F32R else pdt, kind="ExternalInput")
    wvt_d = nc.dram_tensor("wvt", [H, D], F32 if pdt == F32R else pdt, kind="ExternalInput")
    wot_d = nc.dram_tensor("wot", [QD, H], BF16, kind="ExternalInput")
    cos_d = nc.dram_tensor("cosb", [D, T], F32, kind="ExternalInput")
    sin_d = nc.dram_tensor("sinb", [D, T], F32, kind="ExternalInput")
    mask_d = nc.dram_tensor("masks", [4, D, QT], F32, kind="ExternalInput")
    iden_d = nc.dram_tensor("ident", [D, D], F32, kind="ExternalInput")
    ones_d = nc.dram_tensor("ones", [D, D], F32, kind="ExternalInput")
    onesb_d = nc.dram_tensor("onesb", [D, 1], BF16, kind="ExternalInput")
    out_d = nc.dram_tensor("out", [T, H], F32, kind="ExternalOutput")

    def pcast(ap):
        """View a (f32-declared) DRAM AP as f32r when PROJ_DT is f32r."""
        return ap.bitcast(F32R) if pdt == F32R else ap

    HC = H // 128                 # 32 h-chunks
    NPAN = T // PW                # 8 token panels
    NQT = S // QT                 # 8 q-tiles per batch
    QC = QD // 128                # 4 qd chunks == heads per core

    with tile.TileContext(nc) as tc, nc.allow_low_precision(reason="mixed dtypes"):
        with (
            tc.tile_pool(name="persist", bufs=1) as ppool,
            tc.tile_pool(name="dram", bufs=1, space="DRAM") as dpool,
        ):
            # K^T (rope'd) and V natural-layout (bf16) stay in SBUF
            kt_full = ppool.tile([D, T], pdt)
            vnat = ppool.tile([128, T // 128, D], BF16)
            qt_dram = [
                [
                    dpool.tile([D, S], pdt, name=f"qt_b{b}_h{h}", tag=f"qt_b{b}_h{h}")
                    for h in range(HPC)
                ]
                for b in range(B)
            ]

            # ---------------- Phase A: QKV projections + RoPE ----------------
            with (
                tc.tile_pool(name="wpool", bufs=1) as wpool,
                tc.tile_pool(name="xpool", bufs=4) as xpool,
                tc.tile_pool(name="cspool", bufs=1) as cspool,
                tc.tile_pool(name="apool", bufs=2) as apool,
                tc.tile_pool(name="psA", bufs=1, space="PSUM") as psA,
            ):
                wq_s = wpool.tile([128, HC, QD], pdt)
                nc.sync.dma_start(
                    wq_s[:], pcast(wqt_d[:]).rearrange("(hc p) m -> p hc m", p=128)
                )
                wk_s = wpool.tile([128, HC, D], pdt)
                nc.sync.dma_start(
                    wk_s[:], pcast(wkt_d[:]).rearrange("(hc p) m -> p hc m", p=128)
                )
                wv_s = wpool.tile([128, HC, D], pdt)
                nc.sync.dma_start(
                    wv_s[:], pcast(wvt_d[:]).rearrange("(hc p) m -> p hc m", p=128)
                )
                cos_s = cspool.tile([D, T], F32)
                nc.sync.dma_start(cos_s[:], cos_d[:])
                sin_s = cspool.tile([D, T], F32)
                nc.sync.dma_start(sin_s[:], sin_d[:])
                iden_s = cspool.tile([D, D], F32)
                nc.sync.dma_start(iden_s[:], iden_d[:])

                for p in range(NPAN):
                    tok = slice(p * PW, (p + 1) * PW)
                    bp = (p * PW) // S        # batch this panel belongs to
                    ps_q = [
                        psA.tile([128, PW], F32, tag=f"psq{j}", name=f"psq{j}")
                        for j in range(HPC)
                    ]
                    ps_k = psA.tile([128, PW], F32, tag="psk")
                    ps_v = psA.tile([128, PW], F32, tag="psv")
                    for hc in range(HC):
                        x_c = xpool.tile([128, PW], pdt, tag="x_c")
                        nc.sync.dma_start(
                            x_c[:], pcast(xt_d[hc * 128 : (hc + 1) * 128, tok])
                        )
                        st, sp = hc == 0, hc == HC - 1
                        for j in range(HPC):
                            nc.tensor.matmul(
                                ps_q[j][:],
                                wq_s[:, hc, j * 128 : (j + 1) * 128],
                                x_c[:],
                                start=st,
                                stop=sp,
                            )
                        nc.tensor.matmul(ps_k[:], wk_s[:, hc, :], x_c[:], start=st, stop=sp)
                        nc.tensor.matmul(ps_v[:], wv_s[:, hc, :], x_c[:], start=st, stop=sp)

                    # RoPE straight off PSUM: rot(x)[p<64] = -x[p+64]; [p>=64] = x[p-64]
                    def rope_math(ps_t, out_ap):
                        rot = apool.tile([128, PW], F32, tag="rot", name="rot")
                        nc.vector.tensor_scalar_mul(rot[0:64, :], ps_t[64:128, :], -1.0)
                        nc.vector.tensor_copy(rot[64:128, :], ps_t[0:64, :])
                        prod = apool.tile([128, PW], F32, tag="prod", name="prod")
                        nc.vector.tensor_mul(out=prod[:], in0=ps_t[:], in1=cos_s[:, tok])
                        nc.vector.tensor_mul(out=rot[:], in0=rot[:], in1=sin_s[:, tok])
                        nc.vector.tensor_add(out=out_ap, in0=prod[:], in1=rot[:])

                    for j in range(HPC):
                        o = apool.tile([128, PW], pdt, tag="o_q", bufs=3, name="o_q")
                        rope_math(ps_q[j], o[:])
                        lo = p * PW - bp * S
                        nc.sync.dma_start(qt_dram[bp][j][:, lo : lo + PW], o[:])
                    rope_math(ps_k, kt_full[:, tok])

                    # V natural layout via PE transpose (ACT does the psum drain)
                    v_sb = apool.tile([128, PW], F32, tag="v_sb")
                    nc.scalar.copy(v_sb[:], ps_v[:])
                    for blk in range(PW // 128):
                        tp = psA.tile([D, D], F32, tag="tp", bufs=2, name="tp")
                        nc.tensor.transpose(
                            tp[:], v_sb[:, blk * 128 : (blk + 1) * 128], iden_s[:]
                        )
                        nc.vector.tensor_copy(vnat[:, p * (PW // 128) + blk, :], tp[:])

            # ------------- Phase B+C: attention + output projection -------------
            with (
                tc.tile_pool(name="wopool", bufs=1) as wopool,
                tc.tile_pool(name="bpool", bufs=1) as bpool,
                tc.tile_pool(name="qpool", bufs=2) as qpool,
                tc.tile_pool(name="epool", bufs=4) as epool,
                tc.tile_pool(name="npool", bufs=2) as npool,
                tc.tile_pool(name="cxpool", bufs=8) as cxpool,
                tc.tile_pool(name="opool", bufs=4) as opool,
                tc.tile_pool(name="psB", bufs=1, space="PSUM") as psB,
            ):
                mask_s = bpool.tile([D, 4, QT], F32)
                nc.sync.dma_start(mask_s[:], mask_d[:].rearrange("m p q -> p m q"))
                ones_s = bpool.tile([D, D], F32R)
                nc.sync.dma_start(ones_s[:], ones_d[:].bitcast(F32R))
                onesb_s = bpool.tile([D, 1], BF16)
                nc.sync.dma_start(onesb_s[:], onesb_d[:])
                wo_s = wopool.tile([128, QC, H], BF16)
                nc.sync.dma_start(
                    wo_s[:], wot_d[:].rearrange("(qc p) hh -> p qc hh", p=128)
                )

                for b in range(B):
                    for t in range(NQT):
                        qsl = slice(t * QT, (t + 1) * QT)
                        kb_lo = max(0, 2 * t - 8)
                        kbs = list(range(kb_lo, 2 * t + 2))
                        ctx_sbs = []
                        for h in range(HPC):
                            q_t = qpool.tile([D, QT], pdt, tag=f"q_t{h}", name=f"q_t{h}")
                            nc.sync.dma_start(q_t[:], qt_dram[b][h][:, qsl])
                            ctx_ps = psB.tile([D, QT], F32, tag="ctx", bufs=2, name="ctx_ps")
                            den_t = psB.tile([D, QT], F32, tag="db", bufs=2, name="den_t")
                            den_ps = den_t[0:1, :]
                            for i, kb in enumerate(kbs):
                                st, sp = i == 0, i == len(kbs) - 1
                                s_ps = psB.tile([D, QT], F32, tag="sc", bufs=2, name="s_ps")
                                nc.tensor.matmul(
                                    s_ps[:],
                                    kt_full[:, b * S + kb * 128 : b * S + (kb + 1) * 128],
                                    q_t[:],
                                    start=True,
                                    stop=True,
                                )
                                # mask bias: A=[far|inf] B=[0|far] C=[diag|0] D=[inf|diag]
                                mi = None
                                if kb == 2 * t + 1:
                                    mi = 3
                                elif kb == 2 * t:
                                    mi = 2
                                elif kb == 2 * t - 7:
                                    mi = 1
                                elif kb == 2 * t - 8:
                                    mi = 0
                                if mi is not None:
                                    nc.vector.tensor_add(
                                        out=s_ps[:], in0=s_ps[:], in1=mask_s[:, mi, :]
                                    )
                                e_sb = epool.tile([D, QT], BF16, tag="e_sb")
                                nc.scalar.activation(e_sb[:], s_ps[:], AF.Exp)
                                nc.tensor.matmul(
                                    ctx_ps[:], vnat[:, (b * S) // 128 + kb, :], e_sb[:],
                                    start=st, stop=sp,
                                )
                                nc.tensor.matmul(
                                    den_ps, onesb_s[:], e_sb[:],
                                    start=st, stop=sp,
                                )
                            rec32 = npool.tile([1, QT], F32, tag="rec32")
                            nc.vector.reciprocal_approx_fast(rec32[:], den_ps)
                            rec = npool.tile([1, QT], F32R, tag="rec")
                            nc.vector.tensor_copy(rec[:], rec32[:])
                            bc_ps = psB.tile([D, QT], F32, tag="db", bufs=2, name="bc_ps")
                            nc.tensor.matmul(
                                bc_ps[:], ones_s[0:1, :], rec[:], start=True, stop=True
                            )
                            bc_sb = npool.tile([D, QT], F32, tag="bc_sb")
                            nc.vector.tensor_copy(bc_sb[:], bc_ps[:])
                            ctx_sb = cxpool.tile([D, QT], BF16, tag="ctx_sb", name="ctx_sb")
                            nc.vector.tensor_mul(out=ctx_sb[:], in0=ctx_ps[:], in1=bc_sb[:])
                            ctx_sbs.append(ctx_sb)

                        # Output projection for these 256 tokens (2 blocks of 128)
                        for tl in range(QT // 128):
                            tok0 = b * S + t * QT + tl * 128
                            for hb in range(H // 512):
                                ps_o = psB.tile([128, 512], F32, tag="ps_o", bufs=2, name="ps_o")
                                for qc in range(QC):
                                    nc.tensor.matmul(
                                        ps_o[:],
                                        ctx_sbs[qc][:, tl * 128 : (tl + 1) * 128],
                                        wo_s[:, qc, hb * 512 : (hb + 1) * 512],
                                        start=(qc == 0),
                                        stop=(qc == QC - 1),
                                    )
                                o_sb = opool.tile([128, 512], F32, tag="o_sb")
                                if hb % 2 == 0:
                                    nc.vector.tensor_copy(o_sb[:], ps_o[:])
                                else:
                                    nc.scalar.copy(o_sb[:], ps_o[:])
                                nc.sync.dma_start(
                                    out_d[tok0 : tok0 + 128, hb * 512 : (hb + 1) * 512],
                                    o_sb[:],
                                )

    nc.finalize()
    _NC_CACHE = nc
    return nc


def _rope_cache_np(position_ids):
    """cos/sin [D, T] transposed rope cache from actual position ids."""
    inv_freq = 1.0 / (ROPE_THETA ** (np.arange(0, D, 2, dtype=np.float64) / D))
    cos_parts, sin_parts = [], []
    for b in range(B):
        t = np.asarray(position_ids[b], dtype=np.float64)
        freqs = np.outer(t, inv_freq)                    # [S, D/2]
        emb = np.concatenate([freqs, freqs], axis=-1)    # [S, D]
        cos_parts.append(np.cos(emb).T)
        sin_parts.append(np.sin(emb).T)
    cos = np.ascontiguousarray(np.concatenate(cos_parts, axis=1), dtype=np.float32)
    sin = np.ascontiguousarray(np.concatenate(sin_parts, axis=1), dtype=np.float32)
    return cos, sin


def _mask_tiles_np():
    """Composite [4, 128, QT] additive bias tiles in [k, q] layout.

    diag[kl, ql] = 0 if kl <= ql else NEG        (k-block == q-block)
    far[kl, ql]  = 0 if ql <  kl else NEG        (k-block == q-block - 8)
    A=[far | allmask]  B=[0 | far]  C=[diag | 0]  D=[allmask | diag]
    """
    kl = np.arange(128)[:, None]
    ql = np.arange(128)[None, :]
    diag = np.where(kl <= ql, 0.0, NEG).astype(np.float32)
    far = np.where(ql < kl, 0.0, NEG).astype(np.float32)
    zero = np.zeros((128, 128), np.float32)
    full = np.full((128, 128), NEG, np.float32)
    A = np.concatenate([far, full], axis=1)
    Bm = np.concatenate([zero, far], axis=1)
    C = np.concatenate([diag, zero], axis=1)
    Dm = np.concatenate([full, diag], axis=1)
    return np.stack([A, Bm, C, Dm]).astype(np.float32)


def host_in_maps(hidden_states, Wq, Wk, Wv, Wo, position_ids):
    """Shard + pre-layout the full inputs into 8 per-core input maps."""
    hidden_states = np.asarray(hidden_states, dtype=np.float32)
    Wq = np.asarray(Wq, dtype=np.float32)
    Wk = np.asarray(Wk, dtype=np.float32)
    Wv = np.asarray(Wv, dtype=np.float32)
    Wo = np.asarray(Wo, dtype=np.float32)

    ndt = _np_dt(PROJ_DT)
    xt = np.ascontiguousarray(hidden_states.reshape(T, H).T).astype(ndt)
    cos, sin = _rope_cache_np(np.asarray(position_ids))
    masks = _mask_tiles_np()
    ident = np.eye(D, dtype=np.float32)
    ones = np.ones((D, D), dtype=np.float32)
    onesb = np.ones((D, 1), dtype=ml_dtypes.bfloat16)
    qscale = 1.0 / math.sqrt(D)

    in_maps = []
    for c in range(N_CORES):
        wqt = np.ascontiguousarray((Wq[c * QD : (c + 1) * QD, :] * qscale).T).astype(ndt)
        wkt = np.ascontiguousarray(Wk[c * D : (c + 1) * D, :].T).astype(ndt)
        wvt = np.ascontiguousarray(Wv[c * D : (c + 1) * D, :].T).astype(ndt)
        wot = np.ascontiguousarray(Wo[:, c * QD : (c + 1) * QD].T).astype(
            ml_dtypes.bfloat16
        )
        in_maps.append(
            {
                "xt": xt,
                "wqt": wqt,
                "wkt": wkt,
                "wvt": wvt,
                "wot": wot,
                "cosb": cos,
                "sinb": sin,
                "masks": masks,
                "ident": ident,
                "ones": ones,
                "onesb": onesb,
            }
        )
    return in_maps


def kernel(hidden_states, Wq, Wk, Wv, Wo, position_ids):
    nc = build_nc()
    in_maps = host_in_maps(hidden_states, Wq, Wk, Wv, Wo, position_ids)
    res = run_bass_kernel_spmd(nc, in_maps, core_ids=list(range(N_CORES)))
    total = res.results[0]["out"]
    for c in range(1, N_CORES):
        total = total + res.results[c]["out"]
    return np.ascontiguousarray(total.reshape(B, S, H), dtype=np.float32)


# revision 5
# speedup vs baseline: 1.5271x; 1.1347x over previous
"""Mistral sliding-window attention (B=2, S=2048, H=4096, 32 q-heads / 8 kv-heads,
head_dim=128, window=1024) on 8 Trainium2 NeuronCores.

Sharding: tensor-parallel over heads. Core c owns q-heads [4c, 4c+4) and kv-head c:
  Wq rows [512c, 512c+512), Wk/Wv rows [128c, 128c+128), Wo cols [512c, 512c+512).
Each core computes a full-shape partial output (its heads' contribution through
Wo); the host sums the 8 partials (standard TP unshard).

Per-core kernel:
  Phase A: QKV projections from X.T, RoPE fused on the drain (cross-partition
           DVE shifts for rotate_half; 1/sqrt(d) folded into Wq host-side).
           K^T and V (PE-transposed) stay resident in SBUF.
  Phase B+C (merged): per (batch, 256-token q-tile): transposed-layout
           attention for the 4 heads (S^T = K^T^T Q^T, mask-bias add, exp on
           ACT -> bf16 P^T, ctx^T = V^T P^T and denominator ones-row matmul in
           bf16, K=1 broadcast matmul + fast-reciprocal normalize), then
           immediately the output projection for those 256 tokens
           (bf16 x Wo^bf16 -> fp32 partial out).
"""

import math
import sys

sys.path.insert(0, "/opt/trn_rl_repo")

import ml_dtypes
import numpy as np

import concourse.bass as bass
import concourse.mybir as mybir
import concourse.tile as tile
from concourse import bacc
from concourse.bass_utils import run_bass_kernel_spmd

# Problem constants (hardcoded per contract)
B, S, H = 2, 2048, 4096
N_HEADS, N_KV_HEADS, D = 32, 8, 128
WINDOW = 1024
ROPE_THETA = 10000.0
N_CORES = 8
HPC = N_HEADS // N_CORES          # q heads per core = 4
QD = HPC * D                      # per-core q projection dim = 512
T = B * S                         # flattened tokens = 4096

PW = 512                          # phase-A token panel width
QT = 256                          # phase-B query tile width (2 q-blocks)
NEG = -1.0e30

F32 = mybir.dt.float32
F32R = mybir.dt.float32r
BF16 = mybir.dt.bfloat16
AF = mybir.ActivationFunctionType
ALU = mybir.AluOpType

# Projection/scores-path dtype: F32R (precise, 2 PE passes) or BF16 (fast).
PROJ_DT = BF16

_NC_CACHE = None


def _np_dt(dt):
    return ml_dtypes.bfloat16 if dt == BF16 else np.float32


def build_nc():
    """Build (once) the single SPMD Bass program all 8 cores run."""
    global _NC_CACHE
    if _NC_CACHE is not None:
        return _NC_CACHE

    nc = bacc.Bacc(None)

    pdt = PROJ_DT
    xt_d = nc.dram_tensor("xt", [H, T], F32 if pdt == F32R else pdt, kind="ExternalInput")
    wqt_d = nc.dram_tensor("wqt", [H, QD], F32 if pdt == F32R else pdt, kind="ExternalInput")
    wkt_d = nc.dram_tensor("wkt", [H, D], F32 if pdt == F32R else pdt, kind="ExternalInput")
    wvt_d = nc.dram_tensor("wvt", [H, D], F32 if pdt == F32R else pdt, kind="ExternalInput")
    wot_d = nc.dram_tensor("wot", [QD, H], BF16, kind="ExternalInput")
    cos_d = nc.dram_tensor("cosb", [D, T], F32, kind="ExternalInput")
    sin_d = nc.dram_tensor("sinb", [D, T], F32, kind="ExternalInput")
    mask_d = nc.dram_tensor("masks", [2, D, D], F32, kind="ExternalInput")
    iden_d = nc.dram_tensor("ident", [D, D], F32, kind="ExternalInput")
    ones_d = nc.dram_tensor("ones", [D, D], F32, kind="ExternalInput")
    onesb_d = nc.dram_tensor("onesb", [D, 1], BF16, kind="ExternalInput")
    out_d = nc.dram_tensor("out", [T, H], F32, kind="ExternalOutput")

    def pcast(ap):
        """View a (f32-declared) DRAM AP as f32r when PROJ_DT is f32r."""
        return ap.bitcast(F32R) if pdt == F32R else ap

    HC = H // 128                 # 32 h-chunks
    NPAN = T // PW                # 8 token panels
    NQT = S // QT                 # 8 q-tiles per batch
    QC = QD // 128                # 4 qd chunks == heads per core

    with tile.TileContext(nc) as tc, nc.allow_low_precision(reason="mixed dtypes"):
        with (
            tc.tile_pool(name="persist", bufs=1) as ppool,
            tc.tile_pool(name="dram", bufs=1, space="DRAM") as dpool,
        ):
            # K^T (rope'd) and V natural-layout (bf16) stay in SBUF
            kt_full = ppool.tile([D, T], pdt)
            vnat = ppool.tile([128, T // 128, D], BF16)
            qt_dram = [
                [
                    dpool.tile([D, S], pdt, name=f"qt_b{b}_h{h}", tag=f"qt_b{b}_h{h}")
                    for h in range(HPC)
                ]
                for b in range(B)
            ]

            # ---------------- Phase A: QKV projections + RoPE ----------------
            with (
                tc.tile_pool(name="wpool", bufs=1) as wpool,
                tc.tile_pool(name="xpool", bufs=6) as xpool,
                tc.tile_pool(name="cspool", bufs=1) as cspool,
                tc.tile_pool(name="apool", bufs=2) as apool,
                tc.tile_pool(name="psA", bufs=1, space="PSUM") as psA,
            ):
                wq_s = wpool.tile([128, HC, QD], pdt)
                nc.sync.dma_start(
                    wq_s[:], pcast(wqt_d[:]).rearrange("(hc p) m -> p hc m", p=128)
                )
                wk_s = wpool.tile([128, HC, D], pdt)
                nc.gpsimd.dma_start(
                    wk_s[:], pcast(wkt_d[:]).rearrange("(hc p) m -> p hc m", p=128)
                )
                wv_s = wpool.tile([128, HC, D], pdt)
                nc.gpsimd.dma_start(
                    wv_s[:], pcast(wvt_d[:]).rearrange("(hc p) m -> p hc m", p=128)
                )
                cos_s = cspool.tile([D, T], F32)
                nc.scalar.dma_start(cos_s[:], cos_d[:])
                sin_s = cspool.tile([D, T], F32)
                nc.scalar.dma_start(sin_s[:], sin_d[:])
                iden_s = cspool.tile([D, D], F32)
                nc.gpsimd.dma_start(iden_s[:], iden_d[:])

                for p in range(NPAN):
                    tok = slice(p * PW, (p + 1) * PW)
                    bp = (p * PW) // S        # batch this panel belongs to
                    ps_q = [
                        psA.tile([128, PW], F32, tag=f"psq{j}", name=f"psq{j}")
                        for j in range(HPC)
                    ]
                    ps_k = psA.tile([128, PW], F32, tag="psk")
                    ps_v = psA.tile([128, PW], F32, tag="psv")
                    for hc in range(HC):
                        x_c = xpool.tile([128, PW], pdt, tag="x_c")
                        nc.sync.dma_start(
                            x_c[:], pcast(xt_d[hc * 128 : (hc + 1) * 128, tok])
                        )
                        st, sp = hc == 0, hc == HC - 1
                        for j in range(HPC):
                            nc.tensor.matmul(
                                ps_q[j][:],
                                wq_s[:, hc, j * 128 : (j + 1) * 128],
                                x_c[:],
                                start=st,
                                stop=sp,
                            )
                        nc.tensor.matmul(ps_k[:], wk_s[:, hc, :], x_c[:], start=st, stop=sp)
                        nc.tensor.matmul(ps_v[:], wv_s[:, hc, :], x_c[:], start=st, stop=sp)

                    # RoPE straight off PSUM: rot(x)[p<64] = -x[p+64]; [p>=64] = x[p-64]
                    def rope_math(ps_t, out_ap):
                        rot = apool.tile([128, PW], F32, tag="rot", bufs=3, name="rot")
                        nc.scalar.mul(rot[0:64, :], ps_t[64:128, :], -1.0)
                        nc.scalar.copy(rot[64:128, :], ps_t[0:64, :])
                        prod = apool.tile([128, PW], F32, tag="prod", bufs=3, name="prod")
                        nc.vector.tensor_mul(out=prod[:], in0=ps_t[:], in1=cos_s[:, tok])
                        nc.vector.tensor_mul(out=rot[:], in0=rot[:], in1=sin_s[:, tok])
                        nc.vector.tensor_add(out=out_ap, in0=prod[:], in1=rot[:])

                    for j in range(HPC):
                        o = apool.tile([128, PW], pdt, tag="o_q", bufs=4, name="o_q")
                        rope_math(ps_q[j], o[:])
                        lo = p * PW - bp * S
                        nc.sync.dma_start(qt_dram[bp][j][:, lo : lo + PW], o[:])
                    rope_math(ps_k, kt_full[:, tok])

                    # V natural layout via PE transpose (ACT does the psum drain)
                    v_sb = apool.tile([128, PW], F32, tag="v_sb")
                    nc.scalar.copy(v_sb[:], ps_v[:])
                    for blk in range(PW // 128):
                        tp = psA.tile([D, D], F32, tag="tp", bufs=2, name="tp")
                        nc.tensor.transpose(
                            tp[:], v_sb[:, blk * 128 : (blk + 1) * 128], iden_s[:]
                        )
                        nc.vector.tensor_copy(vnat[:, p * (PW // 128) + blk, :], tp[:])

            # ------------- Phase B+C: attention + output projection -------------
            with (
                tc.tile_pool(name="wopool", bufs=1) as wopool,
                tc.tile_pool(name="bpool", bufs=1) as bpool,
                tc.tile_pool(name="qpool", bufs=3) as qpool,
                tc.tile_pool(name="epool", bufs=6) as epool,
                tc.tile_pool(name="npool", bufs=2) as npool,
                tc.tile_pool(name="cxpool", bufs=12) as cxpool,
                tc.tile_pool(name="opool", bufs=6) as opool,
                tc.tile_pool(name="psB", bufs=1, space="PSUM") as psB,
            ):
                mask_s = bpool.tile([D, 2, D], F32)
                nc.gpsimd.dma_start(mask_s[:], mask_d[:].rearrange("m p q -> p m q"))
                ones_s = bpool.tile([D, D], F32R)
                nc.gpsimd.dma_start(ones_s[:], ones_d[:].bitcast(F32R))
                onesb_s = bpool.tile([D, 1], BF16)
                nc.gpsimd.dma_start(onesb_s[:], onesb_d[:])
                wo_s = wopool.tile([128, QC, H], BF16)
                nc.scalar.dma_start(
                    wo_s[:], wot_d[:].rearrange("(qc p) hh -> p qc hh", p=128)
                )

                for b in range(B):
                    for t in range(NQT):
                        qsl = slice(t * QT, (t + 1) * QT)
                        kb_lo = max(0, 2 * t - 8)
                        kbs = list(range(kb_lo, 2 * t + 2))
                        ctx_sbs = []
                        for h in range(HPC):
                            q_t = qpool.tile([D, QT], pdt, tag=f"q_t{h}", name=f"q_t{h}")
                            nc.sync.dma_start(q_t[:], qt_dram[b][h][:, qsl])
                            ctx_ps = psB.tile([D, QT], F32, tag="ctx", bufs=2, name="ctx_ps")
                            den_t = psB.tile([D, QT], F32, tag="db", bufs=2, name="den_t")
                            den_ps = den_t[0:1, :]
                            for i, kb in enumerate(kbs):
                                st, sp = i == 0, i == len(kbs) - 1
                                s_ps = psB.tile([D, QT], F32, tag="sc", bufs=2, name="s_ps")
                                nc.tensor.matmul(
                                    s_ps[:],
                                    kt_full[:, b * S + kb * 128 : b * S + (kb + 1) * 128],
                                    q_t[:],
                                    start=True,
                                    stop=True,
                                )
                                # masks: kb==2t+1 -> diag on right half (left dead)
                                #        kb==2t   -> diag on left half
                                #        kb==2t-7 -> far on right half
                                #        kb==2t-8 -> far on left half (right dead)
                                e_sb = epool.tile([D, QT], BF16, tag="e_sb")
                                lh, rh = slice(0, 128), slice(128, QT)
                                if kb == 2 * t + 1:
                                    nc.vector.tensor_add(
                                        out=s_ps[:, rh], in0=s_ps[:, rh], in1=mask_s[:, 0, :]
                                    )
                                    nc.vector.memset(e_sb[:, lh], 0.0)
                                    nc.scalar.activation(e_sb[:, rh], s_ps[:, rh], AF.Exp)
                                elif kb == 2 * t - 8:
                                    nc.vector.tensor_add(
                                        out=s_ps[:, lh], in0=s_ps[:, lh], in1=mask_s[:, 1, :]
                                    )
                                    nc.vector.memset(e_sb[:, rh], 0.0)
                                    nc.scalar.activation(e_sb[:, lh], s_ps[:, lh], AF.Exp)
                                else:
                                    if kb == 2 * t:
                                        nc.vector.tensor_add(
                                            out=s_ps[:, lh], in0=s_ps[:, lh], in1=mask_s[:, 0, :]
                                        )
                                    elif kb == 2 * t - 7:
                                        nc.vector.tensor_add(
                                            out=s_ps[:, rh], in0=s_ps[:, rh], in1=mask_s[:, 1, :]
                                        )
                                    nc.scalar.activation(e_sb[:], s_ps[:], AF.Exp)
                                nc.tensor.matmul(
                                    ctx_ps[:], vnat[:, (b * S) // 128 + kb, :], e_sb[:],
                                    start=st, stop=sp,
                                )
                                nc.tensor.matmul(
                                    den_ps, onesb_s[:], e_sb[:],
                                    start=st, stop=sp,
                                )
                            rec32 = npool.tile([1, QT], F32, tag="rec32")
                            nc.vector.reciprocal_approx_fast(rec32[:], den_ps)
                            rec = npool.tile([1, QT], F32R, tag="rec")
                            nc.vector.tensor_copy(rec[:], rec32[:])
                            bc_ps = psB.tile([D, QT], F32, tag="db", bufs=2, name="bc_ps")
                            nc.tensor.matmul(
                                bc_ps[:], ones_s[0:1, :], rec[:], start=True, stop=True
                            )
                            bc_sb = npool.tile([D, QT], F32, tag="bc_sb")
                            nc.vector.tensor_copy(bc_sb[:], bc_ps[:])
                            ctx_sb = cxpool.tile([D, QT], BF16, tag="ctx_sb", name="ctx_sb")
                            nc.vector.tensor_mul(out=ctx_sb[:], in0=ctx_ps[:], in1=bc_sb[:])
                            ctx_sbs.append(ctx_sb)

                        # Output projection for these 256 tokens (2 blocks of 128)
                        for tl in range(QT // 128):
                            tok0 = b * S + t * QT + tl * 128
                            for hb in range(H // 512):
                                ps_o = psB.tile([128, 512], F32, tag="ps_o", bufs=2, name="ps_o")
                                for qc in range(QC):
                                    nc.tensor.matmul(
                                        ps_o[:],
                                        ctx_sbs[qc][:, tl * 128 : (tl + 1) * 128],
                                        wo_s[:, qc, hb * 512 : (hb + 1) * 512],
                                        start=(qc == 0),
                                        stop=(qc == QC - 1),
                                    )
                                o_sb = opool.tile([128, 512], F32, tag="o_sb")
                                if hb % 2 == 0:
                                    nc.vector.tensor_copy(o_sb[:], ps_o[:])
                                else:
                                    nc.scalar.copy(o_sb[:], ps_o[:])
                                nc.sync.dma_start(
                                    out_d[tok0 : tok0 + 128, hb * 512 : (hb + 1) * 512],
                                    o_sb[:],
                                )

    nc.finalize()
    _NC_CACHE = nc
    return nc


def _rope_cache_np(position_ids):
    """cos/sin [D, T] transposed rope cache from actual position ids."""
    inv_freq = 1.0 / (ROPE_THETA ** (np.arange(0, D, 2, dtype=np.float64) / D))
    cos_parts, sin_parts = [], []
    for b in range(B):
        t = np.asarray(position_ids[b], dtype=np.float64)
        freqs = np.outer(t, inv_freq)                    # [S, D/2]
        emb = np.concatenate([freqs, freqs], axis=-1)    # [S, D]
        cos_parts.append(np.cos(emb).T)
        sin_parts.append(np.sin(emb).T)
    cos = np.ascontiguousarray(np.concatenate(cos_parts, axis=1), dtype=np.float32)
    sin = np.ascontiguousarray(np.concatenate(sin_parts, axis=1), dtype=np.float32)
    return cos, sin


def _mask_tiles_np():
    """Composite [4, 128, QT] additive bias tiles in [k, q] layout.

    diag[kl, ql] = 0 if kl <= ql else NEG        (k-block == q-block)
    far[kl, ql]  = 0 if ql <  kl else NEG        (k-block == q-block - 8)
    A=[far | allmask]  B=[0 | far]  C=[diag | 0]  D=[allmask | diag]
    """
    kl = np.arange(128)[:, None]
    ql = np.arange(128)[None, :]
    diag = np.where(kl <= ql, 0.0, NEG).astype(np.float32)
    far = np.where(ql < kl, 0.0, NEG).astype(np.float32)
    return np.stack([diag, far]).astype(np.float32)


def host_in_maps(hidden_states, Wq, Wk, Wv, Wo, position_ids):
    """Shard + pre-layout the full inputs into 8 per-core input maps."""
    hidden_states = np.asarray(hidden_states, dtype=np.float32)
    Wq = np.asarray(Wq, dtype=np.float32)
    Wk = np.asarray(Wk, dtype=np.float32)
    Wv = np.asarray(Wv, dtype=np.float32)
    Wo = np.asarray(Wo, dtype=np.float32)

    ndt = _np_dt(PROJ_DT)
    xt = np.ascontiguousarray(hidden_states.reshape(T, H).T).astype(ndt)
    cos, sin = _rope_cache_np(np.asarray(position_ids))
    masks = _mask_tiles_np()
    ident = np.eye(D, dtype=np.float32)
    ones = np.ones((D, D), dtype=np.float32)
    onesb = np.ones((D, 1), dtype=ml_dtypes.bfloat16)
    qscale = 1.0 / math.sqrt(D)

    in_maps = []
    for c in range(N_CORES):
        wqt = np.ascontiguousarray((Wq[c * QD : (c + 1) * QD, :] * qscale).T).astype(ndt)
        wkt = np.ascontiguousarray(Wk[c * D : (c + 1) * D, :].T).astype(ndt)
        wvt = np.ascontiguousarray(Wv[c * D : (c + 1) * D, :].T).astype(ndt)
        wot = np.ascontiguousarray(Wo[:, c * QD : (c + 1) * QD].T).astype(
            ml_dtypes.bfloat16
        )
        in_maps.append(
            {
                "xt": xt,
                "wqt": wqt,
                "wkt": wkt,
                "wvt": wvt,
                "wot": wot,
                "cosb": cos,
                "sinb": sin,
                "masks": masks,
                "ident": ident,
                "ones": ones,
                "onesb": onesb,
            }
        )
    return in_maps


def kernel(hidden_states, Wq, Wk, Wv, Wo, position_ids):
    nc = build_nc()
    in_maps = host_in_maps(hidden_states, Wq, Wk, Wv, Wo, position_ids)
    res = run_bass_kernel_spmd(nc, in_maps, core_ids=list(range(N_CORES)))
    total = res.results[0]["out"]
    for c in range(1, N_CORES):
        total = total + res.results[c]["out"]
    return np.ascontiguousarray(total.reshape(B, S, H), dtype=np.float32)


# revision 6
# speedup vs baseline: 1.6156x; 1.0579x over previous
"""Mistral sliding-window attention (B=2, S=2048, H=4096, 32 q-heads / 8 kv-heads,
head_dim=128, window=1024) on 8 Trainium2 NeuronCores.

Sharding: tensor-parallel over heads. Core c owns q-heads [4c, 4c+4) and kv-head c:
  Wq rows [512c, 512c+512), Wk/Wv rows [128c, 128c+128), Wo cols [512c, 512c+512).
Each core computes a full-shape partial output (its heads' contribution through
Wo); the host sums the 8 partials (standard TP unshard).

Per-core kernel:
  Phase A: QKV projections from X.T, RoPE fused on the drain (cross-partition
           DVE shifts for rotate_half; 1/sqrt(d) folded into Wq host-side).
           K^T and V (PE-transposed) stay resident in SBUF.
  Phase B+C (merged): per (batch, 256-token q-tile): transposed-layout
           attention for the 4 heads (S^T = K^T^T Q^T, mask-bias add, exp on
           ACT -> bf16 P^T, ctx^T = V^T P^T and denominator ones-row matmul in
           bf16, K=1 broadcast matmul + fast-reciprocal normalize), then
           immediately the output projection for those 256 tokens
           (bf16 x Wo^bf16 -> fp32 partial out).
"""

import math
import sys

sys.path.insert(0, "/opt/trn_rl_repo")

import ml_dtypes
import numpy as np

import concourse.bass as bass
import concourse.mybir as mybir
import concourse.tile as tile
from concourse import bacc
from concourse.bass_utils import run_bass_kernel_spmd

# Problem constants (hardcoded per contract)
B, S, H = 2, 2048, 4096
N_HEADS, N_KV_HEADS, D = 32, 8, 128
WINDOW = 1024
ROPE_THETA = 10000.0
N_CORES = 8
HPC = N_HEADS // N_CORES          # q heads per core = 4
QD = HPC * D                      # per-core q projection dim = 512
T = B * S                         # flattened tokens = 4096

PW = 512                          # phase-A token panel width
QT = 256                          # phase-B query tile width (2 q-blocks)
NEG = -1.0e30

F32 = mybir.dt.float32
F32R = mybir.dt.float32r
BF16 = mybir.dt.bfloat16
AF = mybir.ActivationFunctionType
ALU = mybir.AluOpType

# Projection/scores-path dtype: F32R (precise, 2 PE passes) or BF16 (fast).
PROJ_DT = BF16

_NC_CACHE = None


def _np_dt(dt):
    return ml_dtypes.bfloat16 if dt == BF16 else np.float32


def build_nc():
    """Build (once) the single SPMD Bass program all 8 cores run."""
    global _NC_CACHE
    if _NC_CACHE is not None:
        return _NC_CACHE

    nc = bacc.Bacc(None)

    pdt = PROJ_DT
    xt_d = nc.dram_tensor("xt", [H, T], F32 if pdt == F32R else pdt, kind="ExternalInput")
    wqt_d = nc.dram_tensor("wqt", [H, QD], F32 if pdt == F32R else pdt, kind="ExternalInput")
    wkt_d = nc.dram_tensor("wkt", [H, D], F32 if pdt == F32R else pdt, kind="ExternalInput")
    wvt_d = nc.dram_tensor("wvt", [H, D], F32 if pdt == F32R else pdt, kind="ExternalInput")
    wot_d = nc.dram_tensor("wot", [QD, H], BF16, kind="ExternalInput")
    cos_d = nc.dram_tensor("cosb", [D, T], F32, kind="ExternalInput")
    sin_d = nc.dram_tensor("sinb", [D, T], F32, kind="ExternalInput")
    mask_d = nc.dram_tensor("masks", [2, D, D], F32, kind="ExternalInput")
    iden_d = nc.dram_tensor("ident", [D, D], F32, kind="ExternalInput")
    ones_d = nc.dram_tensor("ones", [D, D], F32, kind="ExternalInput")
    onesb_d = nc.dram_tensor("onesb", [D, 1], BF16, kind="ExternalInput")
    out_d = nc.dram_tensor("out", [T, H], F32, kind="ExternalOutput")

    def pcast(ap):
        """View a (f32-declared) DRAM AP as f32r when PROJ_DT is f32r."""
        return ap.bitcast(F32R) if pdt == F32R else ap

    HC = H // 128                 # 32 h-chunks
    NPAN = T // PW                # 8 token panels
    NQT = S // QT                 # 8 q-tiles per batch
    QC = QD // 128                # 4 qd chunks == heads per core

    with tile.TileContext(nc) as tc, nc.allow_low_precision(reason="mixed dtypes"):
        with (
            tc.tile_pool(name="persist", bufs=1) as ppool,
            tc.tile_pool(name="dram", bufs=1, space="DRAM") as dpool,
        ):
            # K^T (rope'd) and V natural-layout (bf16) stay in SBUF
            kt_full = ppool.tile([D, T], pdt)
            vnat = ppool.tile([128, T // 128, D], BF16)
            qt_dram = [
                [
                    dpool.tile([D, S], pdt, name=f"qt_b{b}_h{h}", tag=f"qt_b{b}_h{h}")
                    for h in range(HPC)
                ]
                for b in range(B)
            ]

            # ---------------- Phase A: QKV projections + RoPE ----------------
            with (
                tc.tile_pool(name="wpool", bufs=1) as wpool,
                tc.tile_pool(name="xpool", bufs=6) as xpool,
                tc.tile_pool(name="cspool", bufs=1) as cspool,
                tc.tile_pool(name="apool", bufs=2) as apool,
                tc.tile_pool(name="psA", bufs=1, space="PSUM") as psA,
            ):
                wq_s = wpool.tile([128, HC, QD], pdt)
                nc.sync.dma_start(
                    wq_s[:], pcast(wqt_d[:]).rearrange("(hc p) m -> p hc m", p=128)
                )
                wk_s = wpool.tile([128, HC, D], pdt)
                nc.gpsimd.dma_start(
                    wk_s[:], pcast(wkt_d[:]).rearrange("(hc p) m -> p hc m", p=128)
                )
                wv_s = wpool.tile([128, HC, D], pdt)
                nc.gpsimd.dma_start(
                    wv_s[:], pcast(wvt_d[:]).rearrange("(hc p) m -> p hc m", p=128)
                )
                cos_s = cspool.tile([D, T], F32)
                nc.scalar.dma_start(cos_s[:], cos_d[:])
                sin_s = cspool.tile([D, T], F32)
                nc.scalar.dma_start(sin_s[:], sin_d[:])
                iden_s = cspool.tile([D, D], F32)
                nc.gpsimd.dma_start(iden_s[:], iden_d[:])

                for p in range(NPAN):
                    tok = slice(p * PW, (p + 1) * PW)
                    bp = (p * PW) // S        # batch this panel belongs to
                    ps_q = [
                        psA.tile([128, PW], F32, tag=f"psq{j}", name=f"psq{j}")
                        for j in range(HPC)
                    ]
                    ps_k = psA.tile([128, PW], F32, tag="psk")
                    ps_v = psA.tile([128, PW], F32, tag="psv")
                    for hc in range(HC):
                        x_c = xpool.tile([128, PW], pdt, tag="x_c")
                        nc.sync.dma_start(
                            x_c[:], pcast(xt_d[hc * 128 : (hc + 1) * 128, tok])
                        )
                        st, sp = hc == 0, hc == HC - 1
                        for j in range(HPC):
                            nc.tensor.matmul(
                                ps_q[j][:],
                                wq_s[:, hc, j * 128 : (j + 1) * 128],
                                x_c[:],
                                start=st,
                                stop=sp,
                            )
                        nc.tensor.matmul(ps_k[:], wk_s[:, hc, :], x_c[:], start=st, stop=sp)
                        nc.tensor.matmul(ps_v[:], wv_s[:, hc, :], x_c[:], start=st, stop=sp)

                    # RoPE straight off PSUM: rot(x)[p<64] = -x[p+64]; [p>=64] = x[p-64]
                    def rope_math(ps_t, out_ap):
                        rot = apool.tile([128, PW], F32, tag="rot", bufs=3, name="rot")
                        nc.scalar.mul(rot[0:64, :], ps_t[64:128, :], -1.0)
                        nc.scalar.copy(rot[64:128, :], ps_t[0:64, :])
                        prod = apool.tile([128, PW], F32, tag="prod", bufs=3, name="prod")
                        nc.vector.tensor_mul(out=prod[:], in0=ps_t[:], in1=cos_s[:, tok])
                        nc.vector.tensor_mul(out=rot[:], in0=rot[:], in1=sin_s[:, tok])
                        nc.vector.tensor_add(out=out_ap, in0=prod[:], in1=rot[:])

                    for j in range(HPC):
                        o = apool.tile([128, PW], pdt, tag="o_q", bufs=4, name="o_q")
                        rope_math(ps_q[j], o[:])
                        lo = p * PW - bp * S
                        nc.sync.dma_start(qt_dram[bp][j][:, lo : lo + PW], o[:])
                    rope_math(ps_k, kt_full[:, tok])

                    # V natural layout via PE transpose (ACT does the psum drain)
                    v_sb = apool.tile([128, PW], F32, tag="v_sb")
                    nc.scalar.copy(v_sb[:], ps_v[:])
                    for blk in range(PW // 128):
                        tp = psA.tile([D, D], F32, tag="tp", bufs=2, name="tp")
                        nc.tensor.transpose(
                            tp[:], v_sb[:, blk * 128 : (blk + 1) * 128], iden_s[:]
                        )
                        nc.vector.tensor_copy(vnat[:, p * (PW // 128) + blk, :], tp[:])

            # ------------- Phase B+C: attention + output projection -------------
            with (
                tc.tile_pool(name="wopool", bufs=1) as wopool,
                tc.tile_pool(name="bpool", bufs=1) as bpool,
                tc.tile_pool(name="qpool", bufs=3) as qpool,
                tc.tile_pool(name="epool", bufs=6) as epool,
                tc.tile_pool(name="npool", bufs=2) as npool,
                tc.tile_pool(name="cxpool", bufs=12) as cxpool,
                tc.tile_pool(name="opool", bufs=6) as opool,
                tc.tile_pool(name="psB", bufs=1, space="PSUM") as psB,
            ):
                mask_s = bpool.tile([D, 2, D], F32)
                nc.gpsimd.dma_start(mask_s[:], mask_d[:].rearrange("m p q -> p m q"))
                ones_s = bpool.tile([D, D], F32R)
                nc.gpsimd.dma_start(ones_s[:], ones_d[:].bitcast(F32R))
                onesb_s = bpool.tile([D, 1], BF16)
                nc.gpsimd.dma_start(onesb_s[:], onesb_d[:])
                wo_s = wopool.tile([128, QC, H], BF16)
                nc.scalar.dma_start(
                    wo_s[:], wot_d[:].rearrange("(qc p) hh -> p qc hh", p=128)
                )

                for b in range(B):
                    for t in range(NQT):
                        qsl = slice(t * QT, (t + 1) * QT)
                        kb_lo = max(0, 2 * t - 8)
                        kbs = list(range(kb_lo, 2 * t + 2))
                        ctx_sbs = [None] * HPC
                        for g in range(HPC // 2):      # head pairs (2g, 2g+1)
                            qp = qpool.tile([D, 2 * QT], pdt, tag=f"qp{g}", name=f"qp{g}")
                            nc.sync.dma_start(qp[:, 0:QT], qt_dram[b][2 * g][:, qsl])
                            nc.sync.dma_start(qp[:, QT:], qt_dram[b][2 * g + 1][:, qsl])
                            ctx2 = psB.tile([D, 2 * QT], F32, tag="ctx", bufs=2, name="ctx2")
                            den2_t = psB.tile([D, 2 * QT], F32, tag="db", bufs=2, name="den2_t")
                            den2 = den2_t[0:1, :]
                            for i, kb in enumerate(kbs):
                                st, sp = i == 0, i == len(kbs) - 1
                                s_ps = psB.tile([D, 2 * QT], F32, tag="sc", bufs=2, name="s_ps")
                                nc.tensor.matmul(
                                    s_ps[:],
                                    kt_full[:, b * S + kb * 128 : b * S + (kb + 1) * 128],
                                    qp[:],
                                    start=True,
                                    stop=True,
                                )
                                e2 = epool.tile([D, 2 * QT], BF16, tag="e_sb", name="e2")
                                # masks per head-half: diag on its q-block-diag kb,
                                # far on its window-edge kb; dead halves memset.
                                for hh in range(2):
                                    off = hh * QT
                                    lh = slice(off, off + 128)
                                    rh = slice(off + 128, off + QT)
                                    if kb == 2 * t + 1:
                                        nc.vector.tensor_add(
                                            out=s_ps[:, rh], in0=s_ps[:, rh], in1=mask_s[:, 0, :]
                                        )
                                        nc.vector.memset(e2[:, lh], 0.0)
                                        nc.scalar.activation(e2[:, rh], s_ps[:, rh], AF.Exp)
                                    elif kb == 2 * t - 8:
                                        nc.vector.tensor_add(
                                            out=s_ps[:, lh], in0=s_ps[:, lh], in1=mask_s[:, 1, :]
                                        )
                                        nc.vector.memset(e2[:, rh], 0.0)
                                        nc.scalar.activation(e2[:, lh], s_ps[:, lh], AF.Exp)
                                    elif kb == 2 * t:
                                        nc.vector.tensor_add(
                                            out=s_ps[:, lh], in0=s_ps[:, lh], in1=mask_s[:, 0, :]
                                        )
                                    elif kb == 2 * t - 7:
                                        nc.vector.tensor_add(
                                            out=s_ps[:, rh], in0=s_ps[:, rh], in1=mask_s[:, 1, :]
                                        )
                                if kb not in (2 * t + 1, 2 * t - 8):
                                    nc.scalar.activation(e2[:], s_ps[:], AF.Exp)
                                nc.tensor.matmul(
                                    ctx2[:], vnat[:, (b * S) // 128 + kb, :], e2[:],
                                    start=st, stop=sp,
                                )
                                nc.tensor.matmul(
                                    den2, onesb_s[:], e2[:],
                                    start=st, stop=sp,
                                )
                            for hh in range(2):
                                h = 2 * g + hh
                                hsl = slice(hh * QT, (hh + 1) * QT)
                                rec32 = npool.tile([1, QT], F32, tag="rec32", name="rec32")
                                nc.vector.reciprocal_approx_fast(rec32[:], den2[0:1, hsl])
                                rec = npool.tile([1, QT], F32R, tag="rec", name="rec")
                                nc.vector.tensor_copy(rec[:], rec32[:])
                                bc_t = psB.tile([128, 512], F32, tag="ps_o", bufs=2, name="bc_t")
                                bc_ps = bc_t[:, 0:QT]
                                nc.tensor.matmul(
                                    bc_ps, ones_s[0:1, :], rec[:], start=True, stop=True
                                )
                                bc_sb = npool.tile([D, QT], F32, tag="bc_sb", name="bc_sb")
                                nc.vector.tensor_copy(bc_sb[:], bc_ps)
                                ctx_sb = cxpool.tile([D, QT], BF16, tag="ctx_sb", name="ctx_sb")
                                nc.vector.tensor_mul(out=ctx_sb[:], in0=ctx2[:, hsl], in1=bc_sb[:])
                                ctx_sbs[h] = ctx_sb

                        # Output projection for these 256 tokens (2 blocks of 128)
                        for tl in range(QT // 128):
                            tok0 = b * S + t * QT + tl * 128
                            for hb in range(H // 512):
                                ps_o = psB.tile([128, 512], F32, tag="ps_o", bufs=2, name="ps_o")
                                for qc in range(QC):
                                    nc.tensor.matmul(
                                        ps_o[:],
                                        ctx_sbs[qc][:, tl * 128 : (tl + 1) * 128],
                                        wo_s[:, qc, hb * 512 : (hb + 1) * 512],
                                        start=(qc == 0),
                                        stop=(qc == QC - 1),
                                    )
                                o_sb = opool.tile([128, 512], F32, tag="o_sb")
                                if hb % 2 == 0:
                                    nc.vector.tensor_copy(o_sb[:], ps_o[:])
                                else:
                                    nc.scalar.copy(o_sb[:], ps_o[:])
                                nc.sync.dma_start(
                                    out_d[tok0 : tok0 + 128, hb * 512 : (hb + 1) * 512],
                                    o_sb[:],
                                )

    nc.finalize()
    _NC_CACHE = nc
    return nc


def _rope_cache_np(position_ids):
    """cos/sin [D, T] transposed rope cache from actual position ids."""
    inv_freq = 1.0 / (ROPE_THETA ** (np.arange(0, D, 2, dtype=np.float64) / D))
    cos_parts, sin_parts = [], []
    for b in range(B):
        t = np.asarray(position_ids[b], dtype=np.float64)
        freqs = np.outer(t, inv_freq)                    # [S, D/2]
        emb = np.concatenate([freqs, freqs], axis=-1)    # [S, D]
        cos_parts.append(np.cos(emb).T)
        sin_parts.append(np.sin(emb).T)
    cos = np.ascontiguousarray(np.concatenate(cos_parts, axis=1), dtype=np.float32)
    sin = np.ascontiguousarray(np.concatenate(sin_parts, axis=1), dtype=np.float32)
    return cos, sin


def _mask_tiles_np():
    """Composite [4, 128, QT] additive bias tiles in [k, q] layout.

    diag[kl, ql] = 0 if kl <= ql else NEG        (k-block == q-block)
    far[kl, ql]  = 0 if ql <  kl else NEG        (k-block == q-block - 8)
    A=[far | allmask]  B=[0 | far]  C=[diag | 0]  D=[allmask | diag]
    """
    kl = np.arange(128)[:, None]
    ql = np.arange(128)[None, :]
    diag = np.where(kl <= ql, 0.0, NEG).astype(np.float32)
    far = np.where(ql < kl, 0.0, NEG).astype(np.float32)
    return np.stack([diag, far]).astype(np.float32)


def host_in_maps(hidden_states, Wq, Wk, Wv, Wo, position_ids):
    """Shard + pre-layout the full inputs into 8 per-core input maps."""
    hidden_states = np.asarray(hidden_states, dtype=np.float32)
    Wq = np.asarray(Wq, dtype=np.float32)
    Wk = np.asarray(Wk, dtype=np.float32)
    Wv = np.asarray(Wv, dtype=np.float32)
    Wo = np.asarray(Wo, dtype=np.float32)

    ndt = _np_dt(PROJ_DT)
    xt = np.ascontiguousarray(hidden_states.reshape(T, H).T).astype(ndt)
    cos, sin = _rope_cache_np(np.asarray(position_ids))
    masks = _mask_tiles_np()
    ident = np.eye(D, dtype=np.float32)
    ones = np.ones((D, D), dtype=np.float32)
    onesb = np.ones((D, 1), dtype=ml_dtypes.bfloat16)
    qscale = 1.0 / math.sqrt(D)

    in_maps = []
    for c in range(N_CORES):
        wqt = np.ascontiguousarray((Wq[c * QD : (c + 1) * QD, :] * qscale).T).astype(ndt)
        wkt = np.ascontiguousarray(Wk[c * D : (c + 1) * D, :].T).astype(ndt)
        wvt = np.ascontiguousarray(Wv[c * D : (c + 1) * D, :].T).astype(ndt)
        wot = np.ascontiguousarray(Wo[:, c * QD : (c + 1) * QD].T).astype(
            ml_dtypes.bfloat16
        )
        in_maps.append(
            {
                "xt": xt,
                "wqt": wqt,
                "wkt": wkt,
                "wvt": wvt,
                "wot": wot,
                "cosb": cos,
                "sinb": sin,
                "masks": masks,
                "ident": ident,
                "ones": ones,
                "onesb": onesb,
            }
        )
    return in_maps


def kernel(hidden_states, Wq, Wk, Wv, Wo, position_ids):
    nc = build_nc()
    in_maps = host_in_maps(hidden_states, Wq, Wk, Wv, Wo, position_ids)
    res = run_bass_kernel_spmd(nc, in_maps, core_ids=list(range(N_CORES)))
    total = res.results[0]["out"]
    for c in range(1, N_CORES):
        total = total + res.results[c]["out"]
    return np.ascontiguousarray(total.reshape(B, S, H), dtype=np.float32)
